# revision 2
# baseline (speedup 1.0000x reference)
"""Trainium2 Bass kernel for nn_Attention_87058987090007 — v2 (bf16).

Multi-head attention (B=8, N=1024, D=768, H=12) — data-parallel over batch
across 8 NeuronCores. All matmuls bf16 (fp32 PSUM accumulation), which keeps
max-rel-err ~6e-3 (<< 2e-2 gate).

PE packing (all verified on HW):
  - S^T = K^T Q per head has K(contraction)=64: the two heads of a pair run
    CONCURRENTLY in disjoint row-groups of the PE array (tile_position (0,0)
    and (64,0)) -> 2x on the S phase.
  - AV runs the pair col-packed: V_h0 -> output partitions 0:64, V_h1 ->
    64:128 of ONE [128,1024] accumulator (tile_position (0,0)/(0,64)) -> 2x
    on the AV phase, and the pair shares a single PSUM accumulator.
  - softmax denominators come from M=1 ones-matmuls, four of them packed
    concurrently at col positions 0/32/64/96 of one PSUM bank.
  - reciprocal is ONE [128,512] DVE op per pair (the baseline spent 79us in
    15 single-partition reciprocals).

PSUM budget: S_h0(2) + S_h1(2) + po(2) + rden(1) + mm(1) = 8 banks.
"""

import sys

sys.path.insert(0, "/opt/trn_rl_repo")

import numpy as np
import ml_dtypes

import concourse.bacc as bacc
import concourse.tile as tile
from concourse import mybir
from concourse.bass_utils import run_bass_kernel_spmd

F32 = mybir.dt.float32
BF16 = mybir.dt.bfloat16
I16 = mybir.dt.int16
EXP = mybir.ActivationFunctionType.Exp
ALU = mybir.AluOpType

# Schraudolph fast-exp constants scaled for bf16-bit output via int16
SCH_A16 = 12102203.161561485 / 65536.0
SCH_B16 = 1064866805.0 / 65536.0

CFG = {
    "dve_mts": (),   # mt indices whose exp runs on DVE (Schraudolph approx)
}

B, N, D, H = 8, 1024, 768, 12
d = D // H            # 64 head dim
NT = N // 128         # 8 key tiles
KD = D // 128         # 6 contraction tiles over D
PAIRS = H // 2        # 6 head pairs
CHUNK = 512


def build_nc(reps=1):
    nc = bacc.Bacc(None, target_bir_lowering=False)

    xt = nc.dram_tensor("xt", [D, N], BF16, kind="ExternalInput")
    wqk = nc.dram_tensor("wqk", [2 * PAIRS, 128, D], BF16, kind="ExternalInput")
    wv = nc.dram_tensor("wv", [D, D], BF16, kind="ExternalInput")
    wp = nc.dram_tensor("wp", [D, D], BF16, kind="ExternalInput")
    bp = nc.dram_tensor("bp", [D], F32, kind="ExternalInput")
    y = nc.dram_tensor("y", [N, D], F32, kind="ExternalOutput")
    rbs = [nc.dram_tensor(f"rbs{p}", [128, CHUNK], BF16, kind="Internal")
           for p in range(PAIRS)]

    with tile.TileContext(nc) as tc:
        with (
            tc.tile_pool(name="persist", bufs=1) as persist,
            tc.tile_pool(name="wqkp", bufs=4) as wqkp,
            tc.tile_pool(name="qkp", bufs=6) as qkp,
            tc.tile_pool(name="esp", bufs=2) as esp,
            tc.tile_pool(name="obp", bufs=2) as obp,
            tc.tile_pool(name="rp", bufs=2) as rp,
            tc.tile_pool(name="rbp", bufs=2) as rbp,
            tc.tile_pool(name="yp", bufs=4) as yp,
            tc.tile_pool(name="sp", bufs=1, space="PSUM") as spp,
            tc.tile_pool(name="pop", bufs=1, space="PSUM") as pop,
            tc.tile_pool(name="rdp", bufs=1, space="PSUM") as rdp,
            tc.tile_pool(name="mm", bufs=1, space="PSUM") as psm,
        ):
            for rep in range(reps):
                # ---- persistent loads -------------------------------------
                def load_wqk(m):
                    t = wqkp.tile([128, KD, 128], BF16, name=f"wqk{m}_{rep}",
                                  tag="wqk")
                    nc.gpsimd.dma_start(
                        out=t[:], in_=wqk[m].rearrange("p (k c) -> p k c", k=KD))
                    return t

                wqk0 = load_wqk(0)
                wqk1 = load_wqk(1)

                xts = []
                for k in range(KD):
                    t = persist.tile([128, N], BF16, name=f"xt{k}_{rep}",
                                     tag=f"xt{k}")
                    xts.append(t)
                for c in range(N // CHUNK):
                    for k in range(KD):
                        eng = nc.sync if (k % 2 == 0) else nc.scalar
                        eng.dma_start(
                            out=xts[k][:, c * CHUNK:(c + 1) * CHUNK],
                            in_=xt[k * 128:(k + 1) * 128,
                                   c * CHUNK:(c + 1) * CHUNK])
                wvs = []
                for k in range(KD):
                    t = persist.tile([128, D], BF16, name=f"wv{k}_{rep}",
                                     tag=f"wv{k}")
                    nc.scalar.dma_start(out=t[:], in_=wv[k * 128:(k + 1) * 128, :])
                    wvs.append(t)

                ones = persist.tile([128, 1], BF16, name=f"ones_{rep}", tag="ones")
                nc.vector.memset(ones[:].bitcast(mybir.dt.uint16), 16256)

                # V tiles [128 keys, H, 64] and attention-out (transposed)
                vas = [persist.tile([128, H, d], BF16, name=f"va{t}_{rep}",
                                    tag=f"va{t}") for t in range(NT)]
                aot = [persist.tile([128, N], BF16, name=f"aot{p}_{rep}",
                                    tag=f"aot{p}") for p in range(PAIRS)]

                def qkv_chunk(wtile, dst, c):
                    sl = slice(c * CHUNK, (c + 1) * CHUNK)
                    ps = psm.tile([128, CHUNK], F32, name=f"mmq_{rep}", tag="mm")
                    for k in range(KD):
                        nc.tensor.matmul(ps[:], wtile[:, k, :], xts[k][:, sl],
                                         start=(k == 0), stop=(k == KD - 1))
                    nc.vector.tensor_copy(dst[:, sl], ps[:])

                def qkv_mtile(wtile, dst_name):
                    dst = qkp.tile([128, N], BF16, name=f"{dst_name}_{rep}",
                                   tag="qkc")
                    for c in range(N // CHUNK):
                        qkv_chunk(wtile, dst, c)
                    return dst

                va_dst = ((0, 512), (512, 768))

                def v_tile(t):
                    for lo, hi in va_dst:
                        ps = psm.tile([128, CHUNK], F32, name=f"mmv_{rep}",
                                      tag="mm")[:, :hi - lo]
                        for k in range(KD):
                            nc.tensor.matmul(
                                ps[:], xts[k][:, t * 128:(t + 1) * 128],
                                wvs[k][:, lo:hi],
                                start=(k == 0), stop=(k == KD - 1))
                        nc.vector.tensor_copy(
                            vas[t][:, lo // d:hi // d, :], ps[:])

                wps = []
                for k in range(KD):
                    t = persist.tile([128, D], BF16, name=f"wp{k}_{rep}",
                                     tag=f"wp{k}")
                    nc.gpsimd.dma_start(out=t[:], in_=wp[k * 128:(k + 1) * 128, :])
                    wps.append(t)
                bias = persist.tile([128, D], F32, name=f"bias_{rep}", tag="bias")
                nc.gpsimd.dma_start(out=bias[:], in_=bp[:].partition_broadcast(128))

                # ---- attention, one head pair at a time -------------------
                # Software-pipelined so ACT (exp) runs back-to-back:
                #   per mt: [exp(mt) | S(mt+1) fills during exp | filler qkv/V
                #            | AV(mt)+ones(mt) after exp]
                def s_group(p, mt, qt, kt):
                    mts = slice(mt * 128, (mt + 1) * 128)
                    psa = spp.tile([128, N], F32, name=f"s0_{p}_{mt}_{rep}",
                                   tag="s0")
                    psb = spp.tile([128, N], F32, name=f"s1_{p}_{mt}_{rep}",
                                   tag="s1")
                    for c in range(N // CHUNK):
                        sl = slice(c * CHUNK, (c + 1) * CHUNK)
                        nc.tensor.matmul(psa[:, sl], kt[0:64, mts],
                                         qt[0:64, sl], start=True, stop=True,
                                         tile_position=(0, 0))
                        nc.tensor.matmul(psb[:, sl], kt[64:128, mts],
                                         qt[64:128, sl], start=True,
                                         stop=True, tile_position=(64, 0))
                    return psa, psb

                def attend_pair(p, qt, kt, pss, inline_v=False, next_w=None):
                    h0, h1 = 2 * p, 2 * p + 1
                    po = pop.tile([128, N], F32, name=f"po{p}_{rep}", tag="po")
                    rden = rdp.tile([128, CHUNK], F32, name=f"rd{p}_{rep}",
                                    tag="rd")
                    nxt = []
                    pss_next = None
                    for mt in range(NT):
                        psa, psb = pss
                        es0 = esp.tile([128, N], BF16, name=f"e0_{p}_{mt}_{rep}",
                                       tag="e0")
                        es1 = esp.tile([128, N], BF16, name=f"e1_{p}_{mt}_{rep}",
                                       tag="e1")
                        if mt in CFG["dve_mts"]:
                            nc.vector.tensor_scalar(es0[:].bitcast(I16), psa[:],
                                                    SCH_A16, SCH_B16,
                                                    ALU.mult, ALU.add)
                            nc.vector.tensor_scalar(es1[:].bitcast(I16), psb[:],
                                                    SCH_A16, SCH_B16,
                                                    ALU.mult, ALU.add)
                        else:
                            nc.scalar.activation(es0[:], psa[:], EXP)
                            nc.scalar.activation(es1[:], psb[:], EXP)
                        if mt + 1 < NT:
                            pss = s_group(p, mt + 1, qt, kt)
                        elif next_w is not None:
                            # prefetch next pair's first S tile into the slot
                            # just freed by this exp, so ACT never drains
                            pss_next = s_group(p + 1, 0, nxt[0], nxt[1])
                        if inline_v and mt < len(V_QUEUE):
                            v_chunk(*V_QUEUE[mt])
                        # interleave next pair's qkv projection (fills the PE
                        # while ACT works; emitted BEFORE the es-blocked AV)
                        if next_w is not None and mt % 2 == 1:
                            i = mt // 2
                            if i == 0:
                                nxt.append(qkp.tile([128, N], BF16,
                                                    name=f"qt{p + 1}_{rep}",
                                                    tag="qkc"))
                                nxt.append(qkp.tile([128, N], BF16,
                                                    name=f"kt{p + 1}_{rep}",
                                                    tag="qkc"))
                            qkv_chunk(next_w[i % 2], nxt[i % 2], i // 2)
                        st = (mt == 0)
                        sp_ = (mt == NT - 1)
                        for c in range(N // CHUNK):
                            sl = slice(c * CHUNK, (c + 1) * CHUNK)
                            nc.tensor.matmul(po[64:128, sl], vas[mt][:, h1, :],
                                             es1[:, sl], start=st, stop=sp_,
                                             tile_position=(0, 64))
                            nc.tensor.matmul(po[0:64, sl], vas[mt][:, h0, :],
                                             es0[:, sl], start=st, stop=sp_,
                                             tile_position=(0, 0))
                        nc.tensor.matmul(rden[64:65, :], ones[:], es1[:, 0:CHUNK],
                                         start=st, stop=sp_,
                                         tile_position=(0, 64))
                        nc.tensor.matmul(rden[96:97, :], ones[:],
                                         es1[:, CHUNK:N], start=st, stop=sp_,
                                         tile_position=(0, 96))
                        nc.tensor.matmul(rden[0:1, :], ones[:], es0[:, 0:CHUNK],
                                         start=st, stop=sp_, tile_position=(0, 0))
                        nc.tensor.matmul(rden[32:33, :], ones[:],
                                         es0[:, CHUNK:N], start=st, stop=sp_,
                                         tile_position=(0, 32))

                    # ---- normalize pair -----------------------------------
                    ob = obp.tile([128, N], BF16, name=f"ob{p}_{rep}", tag="ob")
                    nc.vector.tensor_copy(ob[:], po[:])
                    rsb = rp.tile([128, CHUNK], F32, name=f"rs{p}_{rep}",
                                  tag="rs")
                    nc.vector.tensor_copy(rsb[0:97, :], rden[0:97, :])
                    rrec = rp.tile([128, CHUNK], F32, name=f"rr{p}_{rep}",
                                   tag="rr")
                    nc.vector.reciprocal(rrec[0:97, :], rsb[0:97, :])
                    rb16 = rp.tile([128, CHUNK], BF16, name=f"rb16_{p}_{rep}",
                                   tag="rb16")
                    nc.vector.tensor_copy(rb16[0:97, :], rrec[0:97, :])
                    rb = rbp.tile([128, N], BF16, name=f"rb{p}_{rep}", tag="rb")
                    # gpsimd partition_broadcast ignores AP base partitions;
                    # bounce through DRAM where stride-0 partition APs work
                    nc.gpsimd.dma_start(out=rbs[p][:], in_=rb16[:])
                    nc.gpsimd.dma_start(
                        out=rb[0:64, 0:CHUNK],
                        in_=rbs[p][0:1, :].partition_broadcast(64))
                    nc.gpsimd.dma_start(
                        out=rb[0:64, CHUNK:N],
                        in_=rbs[p][32:33, :].partition_broadcast(64))
                    nc.gpsimd.dma_start(
                        out=rb[64:128, 0:CHUNK],
                        in_=rbs[p][64:65, :].partition_broadcast(64))
                    nc.gpsimd.dma_start(
                        out=rb[64:128, CHUNK:N],
                        in_=rbs[p][96:97, :].partition_broadcast(64))
                    nc.vector.tensor_mul(aot[p][:], ob[:], rb[:])
                    return pss_next, nxt

                # pair 0 inputs + first S tile + first V tile
                qt = qkv_mtile(wqk0, "qt0")
                kt = qkv_mtile(wqk1, "kt0")
                pss = s_group(0, 0, qt, kt)
                v_tile(0)

                for p in range(PAIRS):
                    if p + 1 < PAIRS:
                        next_w = [load_wqk(2 * (p + 1)), load_wqk(2 * (p + 1) + 1)]
                    else:
                        next_w = None
                    pss, nxt = attend_pair(p, qt, kt, pss, inline_v=(p == 0),
                                           next_w=next_w)
                    if next_w is not None:
                        qt, kt = nxt[0], nxt[1]

                # ---- output projection ------------------------------------
                for t in range(NT):
                    ys = yp.tile([128, D], F32, name=f"ys{t}_{rep}", tag="ys")
                    for ci, (lo, hi) in enumerate(((0, 512), (512, 768))):
                        j = (2 * t + ci) % 4
                        if j == 0:
                            ps = psm.tile([128, CHUNK], F32, name=f"mmy_{rep}",
                                          tag="mm")[:, :hi - lo]
                        elif j == 1:
                            ps = spp.tile([128, N], F32, name=f"py0_{rep}",
                                          tag="s0")[:, :hi - lo]
                        elif j == 2:
                            ps = spp.tile([128, N], F32, name=f"py1_{rep}",
                                          tag="s1")[:, :hi - lo]
                        else:
                            ps = pop.tile([128, N], F32, name=f"py2_{rep}",
                                          tag="po")[:, :hi - lo]
                        for k in range(KD):
                            nc.tensor.matmul(
                                ps[:], aot[k][:, t * 128:(t + 1) * 128],
                                wps[k][:, lo:hi],
                                start=(k == 0), stop=(k == KD - 1))
                        nc.vector.tensor_add(ys[:, lo:hi], ps[:], bias[:, lo:hi])
                        eng = nc.sync if ci == 0 else nc.scalar
                        eng.dma_start(out=y[t * 128:(t + 1) * 128, lo:hi],
                                      in_=ys[:, lo:hi])

    nc.compile()
    return nc


def prep_inputs(x, Wqkv, Wproj, bproj):
    bf = ml_dtypes.bfloat16
    x = np.asarray(x, dtype=np.float32)
    Wqkv = np.asarray(Wqkv, dtype=np.float32)
    Wproj = np.asarray(Wproj, dtype=np.float32)
    bproj = np.ascontiguousarray(np.asarray(bproj, dtype=np.float32))

    scale = d ** -0.5
    Wq = Wqkv[:, :D] * scale
    Wk = Wqkv[:, D:2 * D]
    Wv = np.ascontiguousarray(Wqkv[:, 2 * D:]).astype(bf)

    wqk = np.empty((2 * PAIRS, 128, D), bf)
    for p in range(PAIRS):
        wqk[2 * p] = (
            Wq[:, p * 128:(p + 1) * 128].reshape(KD, 128, 128)
            .transpose(1, 0, 2).reshape(128, D).astype(bf))
        wqk[2 * p + 1] = (
            Wk[:, p * 128:(p + 1) * 128].reshape(KD, 128, 128)
            .transpose(1, 0, 2).reshape(128, D).astype(bf))

    shared = {"wqk": wqk, "wv": Wv, "wp": Wproj.astype(bf), "bp": bproj}
    in_maps = []
    for b in range(B):
        m = dict(shared)
        m["xt"] = np.ascontiguousarray(x[b].T).astype(bf)
        in_maps.append(m)
    return in_maps


_NC = None


def kernel(x, Wqkv, Wproj, bproj):
    global _NC
    if _NC is None:
        _NC = build_nc()
    in_maps = prep_inputs(x, Wqkv, Wproj, bproj)
    res = run_bass_kernel_spmd(_NC, in_maps, core_ids=list(range(B)))
    return np.stack([res.results[b]["y"] for b in range(B)], axis=0)


if __name__ == "__main__":
    rng = np.random.default_rng(0)
    x = rng.standard_normal((B, N, D), dtype=np.float32)
    Wqkv = rng.standard_normal((D, 3 * D), dtype=np.float32) * D ** -0.5
    Wproj = rng.standard_normal((D, D), dtype=np.float32) * D ** -0.5
    bproj = np.zeros(D, np.float32)
    out = kernel(x=x, Wqkv=Wqkv, Wproj=Wproj, bproj=bproj)
    print("out", out.shape, out.dtype, float(np.abs(out).max()))
